# revision 11
# baseline (speedup 1.0000x reference)
"""Dense-CRF mean-field inference on 8 Trainium2 NeuronCores (v2).

Restructure vs v1 (855937 ns baseline):
  - K = 3*(Kb + Kg) built as K = exp(d2y + d2c + ln3) * Tv + SP3 where
    Tv = exp(-dx^2/50) is an exact Toeplitz x-table (3 variants, 128-px
    blocks repeat mod 3) and SP3 = 3*gy*gx is the host-precomputed
    spatial gaussian (fp16, streamed from HBM during phase 1).
  - d2y + d2color come from ONE bf16 matmul (1 cyc/row vs fp32's 4)
    with hi/lo-split compensated features (19 rows) -> fp32-accurate.
  - K stored fp16; matvec in fp16 (1 cyc/row).  Accuracy is restored by
    a compensated matvec: flat is gathered in fp32, split into fp16
    hi+lo, and K@lo is accumulated over the near band (+-4 rows) only.
  - Per-chunk bands: 3 row-aligned n-chunks (5/5/4 rows), each with its
    own +-20-row m-band (34 blocks) -> 102 hi + 30 lo matmuls/iter.
  - No fat fp32 warm matmuls; early dummy AllGather absorbs the cold
    collective-ring setup (first real gather then runs ~14us warm).
  - Own-block matmuls of the next iteration are emitted before the
    far-block ones so the PE works through them during the gather.

Sharding: core r owns output image rows [12r, 12r+12); K band = global
128-px blocks [9r-16, 9r+25) (zero-K padding outside the image).
Validated on host: rel_err 4.9e-4 (gate 2e-2), see validate.py.
"""

import os
import sys

import numpy as np

for _p in ("/opt/trn_rl_repo",):
    if _p not in sys.path and os.path.isdir(_p):
        sys.path.insert(0, _p)

H = 96
W = 96
C = 5
N = H * W                      # 9216
NCORES = 8
RPC = H // NCORES              # 12 image rows per core
NLOC = (RPC + 2) * W           # 1344 extended-output pixels (14 rows)
NMID = RPC * W                 # 1152 owned pixels
BLK = 128
GBLK = N // BLK                # 72 global blocks
ITERS = 5
LN3 = float(np.log(3.0))
NEG = -1.0e30

# chunk c: ext-local rows [lo_r, hi_r) relative to 12r; ap = cols
CHUNKS = [(-1, 4, 480), (4, 9, 480), (9, 13, 384)]
T_BAND = 19                    # band margin rows
T_COMP = 3                     # compensated (K@flat_lo) margin rows
BAND = [(-15, 18), (-12, 21), (-8, 24)]     # rel block ranges (T=19)
COMP = [(-3, 6), (0, 9), (4, 12)]           # rel block ranges (+-3)
BAND_LO = -15                  # union band start (blocks, rel to 9r)
NBLK_U = 39                    # union band size in blocks
OWN_LO, OWN_HI = 15, 24        # own blocks in band-local coords
NBLKC = [hi - lo for lo, hi in BAND]        # [33, 33, 32]
KW = sum(nb * ap for nb, (_, _, ap) in zip(NBLKC, CHUNKS))   # 43968
CBASE = [0, 33 * 480, 33 * 480 + 33 * 480]  # K col base per chunk
PADBLK = 16
FPW = (GBLK + 2 * PADBLK) * C  # flat_pad cols = 520
NFEAT = 28
HLW = 37                       # weight cols: hi at 0..5, lo at 32..37
ACTB = 4                       # blocks per activation/vector batch

_CACHED_NC = None


def _build_module():
    import concourse.bass as bass
    import concourse.bacc as bacc
    import concourse.tile as tile
    from concourse import mybir
    from concourse.masks import make_identity

    f32 = mybir.dt.float32
    f16 = mybir.dt.float16
    bf16 = mybir.dt.bfloat16
    u32 = mybir.dt.uint32
    EXP = mybir.ActivationFunctionType.Exp
    COPY = mybir.ActivationFunctionType.Copy

    nc = bacc.Bacc("TRN2", target_bir_lowering=False, debug=False,
                   num_devices=NCORES)

    g_dram = nc.dram_tensor("g_feats", [NFEAT, NBLK_U * BLK], bf16,
                            kind="ExternalInput")
    h_dram = nc.dram_tensor("h_feats", [NFEAT, NLOC], bf16,
                            kind="ExternalInput")
    sp3_dram = nc.dram_tensor("sp3", [BLK, KW], f16, kind="ExternalInput")
    ipp_dram = nc.dram_tensor("inp_pp", [BLK, GBLK * C], f32,
                              kind="ExternalInput")
    icn_dram = nc.dram_tensor("inp_cn", [C, NMID], f32, kind="ExternalInput")
    boff_dram = nc.dram_tensor("band_off", [1, 2], u32, kind="ExternalInput")
    out_dram = nc.dram_tensor("out_loc", [BLK, (NMID // BLK) * C], f32,
                              kind="ExternalOutput")

    def bcast_inner(ap, n):
        return bass.AP(tensor=ap.tensor, offset=ap.offset, ap=[*ap.ap, [0, n]])

    with tile.TileContext(nc) as tc:
        with tc.tile_pool(name="singles", bufs=1) as singles, \
             tc.tile_pool(name="dram", bufs=1, space="DRAM") as dram:

            # ---- long-lived SBUF state ----
            kt = [singles.tile([BLK, NBLKC[ci] * CHUNKS[ci][2]], f16,
                               name=f"k{ci}") for ci in range(3)]
            h_sb = singles.tile([NFEAT, NLOC], bf16, name="h_sb")
            g_sb = singles.tile([NFEAT, NBLK_U * BLK], bf16, name="g_sb")
            flat_pad = singles.tile([BLK, FPW], f32, name="flat_pad")
            own32 = singles.tile([BLK, 9 * C], f32, name="own32")
            own_hl = singles.tile([BLK, 9 * HLW], f16, name="own_hl")
            band_hl = singles.tile([BLK, NBLK_U * HLW], f16,
                                   name="band_hl")
            nc.vector.memset(own_hl, 0.0)
            nc.vector.memset(band_hl, 0.0)
            ipp_sb = singles.tile([BLK, GBLK * C], f32, name="ipp_sb")
            icn_sb = singles.tile([C, NMID], f32, name="icn_sb")
            ident = singles.tile([BLK, BLK], f32, name="ident")
            boff_sb = singles.tile([1, 2], u32, name="boff_sb")
            ln3_sb = singles.tile([BLK, 1], f32, name="ln3_sb")
            comb_t1 = singles.tile([C, NLOC], f32, name="t1")
            comb_sb = singles.tile([C, NLOC], f32, name="comb_sb")
            u_cn = singles.tile([C, NMID], f32, name="u_cn")
            u_pp = singles.tile([BLK, 9 * C], f32, name="u_pp")
            nc.vector.memset(ln3_sb, LN3)

            ag_in = dram.tile([BLK, 9 * C], f32, name="ag_in")
            ag_out = nc.dram_tensor("ag_out", [BLK * NCORES, 9 * C], f32,
                                    addr_space="Shared")
            wg_in = dram.tile([BLK, 1], f32, name="wg_in")
            wg_out = nc.dram_tensor("wg_out", [BLK * NCORES, 1], f32,
                                    addr_space="Shared")

            nc.sync.dma_start(out=h_sb, in_=h_dram[:, :])
            GQ = NBLK_U * BLK // 4
            for gq in range(4):
                nc.sync.dma_start(out=g_sb[:, gq * GQ:(gq + 1) * GQ],
                                  in_=g_dram[:, gq * GQ:(gq + 1) * GQ])
            nc.sync.dma_start(out=ipp_sb, in_=ipp_dram[:, :])
            nc.sync.dma_start(out=icn_sb, in_=icn_dram[:, :])
            nc.sync.dma_start(out=boff_sb, in_=boff_dram[:, :])
            make_identity(nc, ident)
            nc.vector.memset(flat_pad, 0.0)

            # warm-up collective: absorbs the cold ring-setup cost (~40us)
            # concurrently with phase 1 so real gathers run warm (~14us).
            nc.sync.dma_start(out=wg_in, in_=ipp_dram[:, 0:1])
            nc.gpsimd.collective_compute(
                "AllGather", mybir.AluOpType.bypass,
                replica_groups=[list(range(NCORES))],
                ins=[wg_in.opt()], outs=[wg_out[:, :]],
            )

            boff_regs = nc.alloc_registers("boff_regs",
                                           engines=(mybir.EngineType.DVE,))
            nc.regs_load(boff_regs, boff_sb[0:1, 0:1])
            off_sv = nc.snap(boff_regs, donate=True, min_val=C,
                             max_val=(NCORES - 1) * 9 * C + C)
            boff2_regs = nc.alloc_registers("boff2_regs",
                                            engines=(mybir.EngineType.DVE,))
            nc.regs_load(boff2_regs, boff_sb[0:1, 1:2])
            own_sv = nc.snap(boff2_regs, donate=True, min_val=0,
                             max_val=(NCORES - 1) * 9 * C)

            # ---- helpers ----
            def softmax_pp(pool, u_ppv, mb, tag, out=None):
                v = u_ppv.rearrange("p (a c) -> p a c", c=C)
                mx = pool.tile([BLK, mb], f32, tag=f"{tag}_mx")
                nc.vector.tensor_reduce(out=mx, in_=v,
                                        axis=mybir.AxisListType.X,
                                        op=mybir.AluOpType.max)
                e = pool.tile([BLK, mb * C], f32, tag=f"{tag}_e")
                ev = e.rearrange("p (a c) -> p a c", c=C)
                nc.vector.tensor_sub(ev, v, bcast_inner(mx, C))
                nc.scalar.activation(out=e, in_=e, func=EXP)
                s = pool.tile([BLK, mb], f32, tag=f"{tag}_s")
                nc.vector.tensor_reduce(out=s, in_=ev,
                                        axis=mybir.AxisListType.X,
                                        op=mybir.AluOpType.add)
                nc.vector.reciprocal(out=s, in_=s)
                if out is None:
                    out = pool.tile([BLK, mb * C], f32, tag=f"{tag}_fl")
                nc.vector.tensor_mul(out.rearrange("p (a c) -> p a c", c=C),
                                     ev, bcast_inner(s, C))
                return out

            # ---- phase 2: initial flat = softmax(input), replicated ----
            with tc.tile_pool(name="init", bufs=1) as ipool:
                fl0 = softmax_pp(ipool, ipp_sb, GBLK, "sm0")
                nc.vector.tensor_copy(
                    out=flat_pad[:, PADBLK * C:(PADBLK + GBLK) * C], in_=fl0)
                nc.vector.tensor_copy(out=own32,
                                      in_=fl0[:, bass.ds(own_sv, 9 * C)])
                ohl = own_hl.rearrange("p (b t) -> p b t", t=HLW)
                o32v = own32.rearrange("p (b c) -> p b c", c=C)
                nc.vector.tensor_copy(out=ohl[:, :, 0:C], in_=o32v)
                nc.vector.tensor_sub(ohl[:, :, 32:32 + C], o32v,
                                     ohl[:, :, 0:C])

            # ---- phase 1: build K band (fp16) ----
            with tc.tile_pool(name="sp3p", bufs=2) as sp3pool, \
                 tc.tile_pool(name="ep", bufs=2) as epool, \
                 tc.tile_pool(name="p1ps", bufs=2, space="PSUM") as p1pool:
                for ci in range(3):
                    lo_b, hi_b = BAND[ci]
                    nb, ap = NBLKC[ci], CHUNKS[ci][2]
                    e0 = (CHUNKS[ci][0] + 1) * W
                    sp3_sb = sp3pool.tile([BLK, 34 * 480], f16, tag="sp3")
                    spq = [0, 8 * ap, 16 * ap, 24 * ap, nb * ap]
                    for q in range(4):
                        nc.sync.dma_start(
                            out=sp3_sb[:, spq[q]:spq[q + 1]],
                            in_=sp3_dram[:, CBASE[ci] + spq[q]:
                                         CBASE[ci] + spq[q + 1]])
                    ktv = kt[ci].rearrange("p (j a) -> p j a", a=ap)
                    sp3v = sp3_sb[:, 0:nb * ap].rearrange(
                        "p (j a) -> p j a", a=ap)
                    for j0 in range(0, nb, ACTB):
                        nj = min(ACTB, nb - j0)
                        pb = p1pool.tile([BLK, ACTB, 512], f32, tag="pb")
                        for jj in range(nj):
                            bi = lo_b - BAND_LO + j0 + jj
                            nc.tensor.matmul(
                                pb[:, jj, 0:ap],
                                g_sb[:, bi * BLK:(bi + 1) * BLK],
                                h_sb[:, e0:e0 + ap],
                                start=True, stop=True)
                        # E = exp(d2y+d2x+d2c+ln3) straight into K
                        nc.scalar.activation(out=ktv[:, j0:j0 + nj, :],
                                             in_=pb[:, 0:nj, 0:ap],
                                             func=EXP, bias=ln3_sb)
                        # K += SP3 (precomputed 3*gy*gx, fp16)
                        nc.vector.tensor_add(ktv[:, j0:j0 + nj, :],
                                             ktv[:, j0:j0 + nj, :],
                                             sp3v[:, j0:j0 + nj, :])

            # ---- phase 3: iterations ----
            with tc.tile_pool(name="smx", bufs=2) as spool, \
                 tc.tile_pool(name="ipsum", bufs=1, space="PSUM") as ippool, \
                 tc.tile_pool(name="tpsum", bufs=2, space="PSUM") as tppool:
                pv = [ippool.tile([HLW, 512], f32, tag=f"pv{ci}",
                                  name=f"pv{ci}") for ci in range(3)]

                def src(bi):
                    if OWN_LO <= bi < OWN_HI:
                        o = bi - OWN_LO
                        return own_hl[:, o * HLW:(o + 1) * HLW]
                    return band_hl[:, bi * HLW:(bi + 1) * HLW]

                ktvs = [kt[ci].rearrange("p (j a) -> p j a",
                                         a=CHUNKS[ci][2]) for ci in range(3)]

                def chunks_for(b):
                    return [ci for ci in range(3)
                            if BAND[ci][0] <= b < BAND[ci][1]]

                for it in range(ITERS):
                    tot = [BAND[ci][1] - BAND[ci][0] for ci in range(3)]
                    cnt = [0, 0, 0]

                    def emit_mm(ci, b):
                        bi = b - BAND_LO
                        j = b - BAND[ci][0]
                        ap = CHUNKS[ci][2]
                        cnt[ci] += 1
                        nc.tensor.matmul(pv[ci][:, 0:ap], src(bi),
                                         ktvs[ci][:, j, :],
                                         start=cnt[ci] == 1,
                                         stop=cnt[ci] == tot[ci])

                    # own blocks first (data ready pre-gather)
                    for b in range(0, 9):
                        for ci in chunks_for(b):
                            emit_mm(ci, b)
                    # band tiles from gathered flat (stalls until scatter)
                    bhl = band_hl.rearrange("p (b t) -> p b t", t=HLW)
                    pdv = flat_pad[:, bass.ds(off_sv, NBLK_U * C)].rearrange(
                        "p (b c) -> p b c", c=C)
                    nc.vector.tensor_copy(out=bhl[:, :, 0:C], in_=pdv)
                    nc.vector.tensor_sub(bhl[:, :, 32:32 + C], pdv,
                                         bhl[:, :, 0:C])
                    # far blocks chunk-sequential: chunk ci's psum closes
                    # while ci+1's matmuls still run -> x-pass overlaps PE
                    for ci in range(3):
                        lo_r, hi_r, ap = CHUNKS[ci]
                        for b in range(BAND[ci][0], BAND[ci][1]):
                            if not (0 <= b < 9):
                                emit_mm(ci, b)
                        # PSUM -> SBUF (scalar), merge lo rows, then x-pass
                        e0 = (lo_r + 1) * W
                        cb = comb_sb[:, e0:e0 + ap]
                        nc.scalar.activation(out=cb, in_=pv[ci][0:C, 0:ap],
                                             func=COPY)
                        nc.vector.tensor_add(cb, cb,
                                             pv[ci][32:32 + C, 0:ap])
                        t1c = comb_t1[:, e0:e0 + ap]
                        nc.vector.tensor_add(t1c[:, 1:ap - 1], cb[:, 0:ap - 2],
                                             cb[:, 2:ap])
                        nc.vector.tensor_add(t1c[:, 1:ap - 1], t1c[:, 1:ap - 1],
                                             cb[:, 1:ap - 1])
                        t1r = t1c.rearrange("p (r x) -> p r x", x=W)
                        cbr = cb.rearrange("p (r x) -> p r x", x=W)
                        nc.vector.tensor_add(t1r[:, :, 0:1], cbr[:, :, 0:1],
                                             cbr[:, :, 1:2])
                        nc.vector.tensor_add(t1r[:, :, 0:1], t1r[:, :, 0:1],
                                             cbr[:, :, 0:1])
                        nc.vector.tensor_add(t1r[:, :, W - 1:W],
                                             cbr[:, :, W - 2:W - 1],
                                             cbr[:, :, W - 1:W])
                        nc.vector.tensor_add(t1r[:, :, W - 1:W],
                                             t1r[:, :, W - 1:W],
                                             cbr[:, :, W - 1:W])
                        # y-pass A after chunk 1 (u rows 0..7, DVE only —
                        # transposes are emitted after far-c2 so they do
                        # not block chunk 2's matmuls on the PE queue)
                        if ci == 1:
                            nc.vector.tensor_add(u_cn[:, 0:768],
                                                 comb_t1[:, 0:768],
                                                 comb_t1[:, W:768 + W])
                            nc.vector.tensor_add(u_cn[:, 0:768],
                                                 u_cn[:, 0:768],
                                                 comb_t1[:, 2 * W:768 + 2 * W])
                            nc.vector.tensor_add(u_cn[:, 0:768],
                                                 u_cn[:, 0:768],
                                                 icn_sb[:, 0:768])
                    # y-pass B (u rows 8..11), then all 9 transposes
                    nc.vector.tensor_add(u_cn[:, 768:NMID],
                                         comb_t1[:, 768:NMID],
                                         comb_t1[:, 768 + W:NMID + W])
                    nc.vector.tensor_add(u_cn[:, 768:NMID], u_cn[:, 768:NMID],
                                         comb_t1[:, 768 + 2 * W:NMID + 2 * W])
                    nc.vector.tensor_add(u_cn[:, 768:NMID], u_cn[:, 768:NMID],
                                         icn_sb[:, 768:NMID])
                    for kb in range(9):
                        pt = tppool.tile([BLK, C], f32, tag="pt")
                        nc.tensor.transpose(pt,
                                            u_cn[:, kb * BLK:(kb + 1) * BLK],
                                            ident[0:C, 0:C])
                        nc.vector.tensor_copy(
                            out=u_pp[:, kb * C:(kb + 1) * C], in_=pt)

                    # softmax split: blocks 0-5 ready before 6-8
                    softmax_pp(spool, u_pp[:, 0:6 * C], 6, "smxA",
                               out=own32[:, 0:6 * C])
                    softmax_pp(spool, u_pp[:, 6 * C:9 * C], 3, "smxB",
                               out=own32[:, 6 * C:9 * C])

                    if it < ITERS - 1:
                        nc.sync.dma_start(out=ag_in, in_=own32)
                        nc.gpsimd.collective_compute(
                            "AllGather", mybir.AluOpType.bypass,
                            replica_groups=[list(range(NCORES))],
                            ins=[ag_in.opt()], outs=[ag_out[:, :]],
                        )
                        nc.sync.dma_start(
                            out=flat_pad[:, PADBLK * C:(PADBLK + GBLK) * C]
                            .rearrange("p (r j) -> p r j", r=NCORES),
                            in_=ag_out[:, :].rearrange("(r p) j -> p r j",
                                                       p=BLK))
                    else:
                        nc.sync.dma_start(out=out_dram[:, :], in_=own32)
                    # own hi/lo for the NEXT iter: off the pre-gather path
                    ohl2 = own_hl.rearrange("p (b t) -> p b t", t=HLW)
                    o32v2 = own32.rearrange("p (b c) -> p b c", c=C)
                    nc.vector.tensor_copy(out=ohl2[:, :, 0:C], in_=o32v2)
                    nc.vector.tensor_sub(ohl2[:, :, 32:32 + C], o32v2,
                                         ohl2[:, :, 0:C])

    nc.compile()
    return nc


def _host_inputs(input_tensor, reference_tensor):
    import ml_dtypes
    bf = ml_dtypes.bfloat16

    logits = np.ascontiguousarray(
        np.asarray(input_tensor, dtype=np.float32)[0].reshape(C, N))
    ref = np.asarray(reference_tensor, dtype=np.float32)[0]  # [3, 96, 96]

    yy = (np.arange(N) // W).astype(np.float64)
    xx = (np.arange(N) % W).astype(np.float64)
    xp = xx - 47.5
    cc = ref.reshape(3, N).astype(np.float64) / 0.5
    ones = np.ones(N, np.float64)

    def hi_lo(x):
        h = np.asarray(x, np.float64).astype(bf).astype(np.float64)
        l = (np.asarray(x, np.float64) - h).astype(bf).astype(np.float64)
        return h, l

    def hi_lo3(x):
        x = np.asarray(x, np.float64)
        h = x.astype(bf).astype(np.float64)
        l = (x - h).astype(bf).astype(np.float64)
        l2 = (x - h - l).astype(bf).astype(np.float64)
        return h, l, l2

    # feature rows (G paired with H), d2 = -(dy^2)/50 - 0.5*|dc~|^2
    def feat_rows(yp):
        rows_G, rows_H = [], []

        def pair(g, h):
            rows_G.append(np.asarray(g, np.float64))
            rows_H.append(np.asarray(h, np.float64))

        g_h, g_l = hi_lo(-yp * yp / 50.0)
        pair(g_h, ones); pair(g_l, ones)
        h_h, h_l = hi_lo(yp / 25.0)
        pair(yp, h_h); pair(yp, h_l)
        h_h, h_l = hi_lo(-yp * yp / 50.0)
        pair(ones, h_h); pair(ones, h_l)
        a, b, c2_ = hi_lo3(-xp * xp / 50.0)
        pair(a, ones); pair(b, ones); pair(c2_, ones)
        a, b, c2_ = hi_lo3(xp / 25.0)
        pair(xp, a); pair(xp, b); pair(xp, c2_)
        a, b, c2_ = hi_lo3(-xp * xp / 50.0)
        pair(ones, a); pair(ones, b); pair(ones, c2_)
        for ch in range(3):
            cm_h, cm_l = hi_lo(cc[ch])
            pair(cm_h, cm_h); pair(cm_h, cm_l); pair(cm_l, cm_h)
        csq = -0.5 * (cc * cc).sum(axis=0)
        g_h, g_l = hi_lo(csq)
        pair(g_h, ones); pair(g_l, ones)
        pair(ones, g_h); pair(ones, g_l)
        return np.stack(rows_G), np.stack(rows_H)  # [19, N] each

    dtab = np.exp(-(np.arange(-(H + 32), H + 32) ** 2) / 50.0)
    yy_i = (np.arange(N) // W).astype(np.int64)
    xx_i = (np.arange(N) % W).astype(np.int64)

    # ipp: logits in pixel-partition layout [128, 72*5]
    ipp = np.ascontiguousarray(
        logits.reshape(C, GBLK, BLK).transpose(2, 1, 0).reshape(BLK, GBLK * C))

    in_maps = []
    for r in range(NCORES):
        yc = 12 * r + 6
        G_all, H_all = feat_rows(yy - yc)
        yext = np.clip(np.arange(RPC * r - 1, RPC * (r + 1) + 1), 0, H - 1)
        hpix = (yext[:, None] * W + np.arange(W)[None, :]).reshape(-1)

        g = np.zeros((NFEAT, NBLK_U * BLK), np.float64)
        g[0, :] = NEG
        for bi in range(NBLK_U):
            gb = 9 * r + BAND_LO + bi
            if 0 <= gb < GBLK:
                g[:, bi * BLK:(bi + 1) * BLK] = \
                    G_all[:, gb * BLK:(gb + 1) * BLK]
        h = H_all[:, hpix]

        sp3 = np.zeros((BLK, KW), np.float16)
        for ci, (lo_r, hi_r, ap) in enumerate(CHUNKS):
            nrows = hi_r - lo_r
            yn = yext[(lo_r + 1):(lo_r + 1) + nrows]
            for j in range(NBLKC[ci]):
                gb = 9 * r + BAND[ci][0] + j
                if not (0 <= gb < GBLK):
                    continue
                pm = np.arange(gb * BLK, (gb + 1) * BLK)
                A = 3.0 * dtab[yy_i[pm][:, None] - yn[None, :] + H + 32]
                B = dtab[xx_i[pm][:, None] - np.arange(W)[None, :] + H + 32]
                blkv = (A[:, :, None] * B[:, None, :]).reshape(BLK, ap)
                c0 = CBASE[ci] + j * ap
                sp3[:, c0:c0 + ap] = blkv.astype(np.float16)

        icn = np.ascontiguousarray(
            logits.reshape(C, H, W)[:, RPC * r:RPC * (r + 1), :]
            .reshape(C, NMID))
        in_maps.append({
            "g_feats": np.ascontiguousarray(g).astype(bf),
            "h_feats": np.ascontiguousarray(h).astype(bf),
            "sp3": sp3,
            "inp_pp": ipp,
            "inp_cn": icn,
            "band_off": np.array([[(PADBLK + 9 * r + BAND_LO) * C, 9 * C * r]], np.uint32),
        })
    return in_maps


def _assemble(results):
    out = np.empty((C, N), np.float32)
    for r in range(NCORES):
        blk = results[r]["out_loc"].reshape(BLK, NMID // BLK, C)
        out[:, NMID * r:NMID * (r + 1)] = (
            blk.transpose(2, 1, 0).reshape(C, NMID))
    return out.reshape(1, C, H, W)


def _get_nc():
    global _CACHED_NC
    if _CACHED_NC is None:
        _CACHED_NC = _build_module()
    return _CACHED_NC


def run(input_tensor, reference_tensor, trace=False):
    from concourse.bass_utils import run_bass_kernel_spmd
    nc = _get_nc()
    in_maps = _host_inputs(input_tensor, reference_tensor)
    res = run_bass_kernel_spmd(nc, in_maps, core_ids=list(range(NCORES)),
                               trace=trace)
    return _assemble(res.results), res


def kernel(input_tensor, reference_tensor):
    out, _ = run(input_tensor, reference_tensor, trace=False)
    return out


# revision 12
# speedup vs baseline: 1.0606x; 1.0606x over previous
"""Dense-CRF mean-field inference on 8 Trainium2 NeuronCores (v2).

Restructure vs v1 (855937 ns baseline):
  - K = 3*(Kb + Kg) built as K = exp(d2y + d2c + ln3) * Tv + SP3 where
    Tv = exp(-dx^2/50) is an exact Toeplitz x-table (3 variants, 128-px
    blocks repeat mod 3) and SP3 = 3*gy*gx is the host-precomputed
    spatial gaussian (fp16, streamed from HBM during phase 1).
  - d2y + d2color come from ONE bf16 matmul (1 cyc/row vs fp32's 4)
    with hi/lo-split compensated features (19 rows) -> fp32-accurate.
  - K stored fp16; matvec in fp16 (1 cyc/row).  Accuracy is restored by
    a compensated matvec: flat is gathered in fp32, split into fp16
    hi+lo, and K@lo is accumulated over the near band (+-4 rows) only.
  - Per-chunk bands: 3 row-aligned n-chunks (5/5/4 rows), each with its
    own +-20-row m-band (34 blocks) -> 102 hi + 30 lo matmuls/iter.
  - No fat fp32 warm matmuls; early dummy AllGather absorbs the cold
    collective-ring setup (first real gather then runs ~14us warm).
  - Own-block matmuls of the next iteration are emitted before the
    far-block ones so the PE works through them during the gather.

Sharding: core r owns output image rows [12r, 12r+12); K band = global
128-px blocks [9r-16, 9r+25) (zero-K padding outside the image).
Validated on host: rel_err 4.9e-4 (gate 2e-2), see validate.py.
"""

import os
import sys

import numpy as np

for _p in ("/opt/trn_rl_repo",):
    if _p not in sys.path and os.path.isdir(_p):
        sys.path.insert(0, _p)

H = 96
W = 96
C = 5
N = H * W                      # 9216
NCORES = 8
RPC = H // NCORES              # 12 image rows per core
NLOC = (RPC + 2) * W           # 1344 extended-output pixels (14 rows)
NMID = RPC * W                 # 1152 owned pixels
BLK = 128
GBLK = N // BLK                # 72 global blocks
ITERS = 5
LN3 = float(np.log(3.0))
NEG = -1.0e30

# chunk c: ext-local rows [lo_r, hi_r) relative to 12r; ap = cols
CHUNKS = [(-1, 4, 480), (4, 9, 480), (9, 13, 384)]
T_BAND = 19                    # band margin rows
T_COMP = 3                     # compensated (K@flat_lo) margin rows
BAND = [(-15, 18), (-12, 21), (-8, 24)]     # rel block ranges (T=19)
COMP = [(-3, 6), (0, 9), (4, 12)]           # rel block ranges (+-3)
BAND_LO = -15                  # union band start (blocks, rel to 9r)
NBLK_U = 39                    # union band size in blocks
OWN_LO, OWN_HI = 15, 24        # own blocks in band-local coords
NBLKC = [hi - lo for lo, hi in BAND]        # [33, 33, 32]
KW = sum(nb * ap for nb, (_, _, ap) in zip(NBLKC, CHUNKS))   # 43968
CBASE = [0, 33 * 480, 33 * 480 + 33 * 480]  # K col base per chunk
PADBLK = 16
FPW = (GBLK + 2 * PADBLK) * C  # flat_pad cols = 520
NFEAT = 28
HLW = 37                       # weight cols: hi at 0..5, lo at 32..37
ACTB = 4                       # blocks per activation/vector batch

_CACHED_NC = None


def _build_module():
    import concourse.bass as bass
    import concourse.bacc as bacc
    import concourse.tile as tile
    from concourse import mybir
    from concourse.masks import make_identity

    f32 = mybir.dt.float32
    f16 = mybir.dt.float16
    bf16 = mybir.dt.bfloat16
    u32 = mybir.dt.uint32
    EXP = mybir.ActivationFunctionType.Exp
    COPY = mybir.ActivationFunctionType.Copy

    nc = bacc.Bacc("TRN2", target_bir_lowering=False, debug=False,
                   num_devices=NCORES)

    g_dram = nc.dram_tensor("g_feats", [NFEAT, NBLK_U * BLK], bf16,
                            kind="ExternalInput")
    h_dram = nc.dram_tensor("h_feats", [NFEAT, NLOC], bf16,
                            kind="ExternalInput")
    sp3_dram = nc.dram_tensor("sp3", [BLK, KW], f16, kind="ExternalInput")
    ipp_dram = nc.dram_tensor("inp_pp", [BLK, GBLK * C], f32,
                              kind="ExternalInput")
    icn_dram = nc.dram_tensor("inp_cn", [C, NMID], f32, kind="ExternalInput")
    boff_dram = nc.dram_tensor("band_off", [1, 2], u32, kind="ExternalInput")
    out_dram = nc.dram_tensor("out_loc", [BLK, (NMID // BLK) * C], f32,
                              kind="ExternalOutput")

    def bcast_inner(ap, n):
        return bass.AP(tensor=ap.tensor, offset=ap.offset, ap=[*ap.ap, [0, n]])

    with tile.TileContext(nc) as tc:
        with tc.tile_pool(name="singles", bufs=1) as singles, \
             tc.tile_pool(name="dram", bufs=1, space="DRAM") as dram:

            # ---- long-lived SBUF state ----
            kt = [singles.tile([BLK, NBLKC[ci] * CHUNKS[ci][2]], f16,
                               name=f"k{ci}") for ci in range(3)]
            h_sb = singles.tile([NFEAT, NLOC], bf16, name="h_sb")
            g_sb = singles.tile([NFEAT, NBLK_U * BLK], bf16, name="g_sb")
            flat_pad = singles.tile([BLK, FPW], f32, name="flat_pad")
            own32 = singles.tile([BLK, 9 * C], f32, name="own32")
            own_hl = singles.tile([BLK, 9 * HLW], f16, name="own_hl")
            band_hl = singles.tile([BLK, NBLK_U * HLW], f16,
                                   name="band_hl")
            nc.vector.memset(own_hl, 0.0)
            nc.vector.memset(band_hl, 0.0)
            ipp_sb = singles.tile([BLK, GBLK * C], f32, name="ipp_sb")
            icn_sb = singles.tile([C, NMID], f32, name="icn_sb")
            ident = singles.tile([BLK, BLK], f32, name="ident")
            boff_sb = singles.tile([1, 2], u32, name="boff_sb")
            ln3_sb = singles.tile([BLK, 1], f32, name="ln3_sb")
            comb_t1 = singles.tile([C, NLOC], f32, name="t1")
            comb_sb = singles.tile([C, NLOC], f32, name="comb_sb")
            u_cn = singles.tile([C, NMID], f32, name="u_cn")
            u_pp = singles.tile([BLK, 9 * C], f32, name="u_pp")
            nc.vector.memset(ln3_sb, LN3)

            ag_in = dram.tile([BLK, 9 * C], f32, name="ag_in")
            ag_out = nc.dram_tensor("ag_out", [BLK * NCORES, 9 * C], f32,
                                    addr_space="Shared")
            wg_in = dram.tile([BLK, 1], f32, name="wg_in")
            wg_out = nc.dram_tensor("wg_out", [BLK * NCORES, 1], f32,
                                    addr_space="Shared")

            nc.sync.dma_start(out=h_sb, in_=h_dram[:, :])
            GQ = NBLK_U * BLK // 4
            for gq in range(4):
                nc.sync.dma_start(out=g_sb[:, gq * GQ:(gq + 1) * GQ],
                                  in_=g_dram[:, gq * GQ:(gq + 1) * GQ])
            nc.sync.dma_start(out=ipp_sb, in_=ipp_dram[:, :])
            nc.sync.dma_start(out=icn_sb, in_=icn_dram[:, :])
            nc.sync.dma_start(out=boff_sb, in_=boff_dram[:, :])
            make_identity(nc, ident)
            nc.vector.memset(flat_pad, 0.0)

            # warm-up collective: absorbs the cold ring-setup cost (~40us)
            # concurrently with phase 1 so real gathers run warm (~14us).
            nc.sync.dma_start(out=wg_in, in_=ipp_dram[:, 0:1])
            nc.gpsimd.collective_compute(
                "AllGather", mybir.AluOpType.bypass,
                replica_groups=[list(range(NCORES))],
                ins=[wg_in.opt()], outs=[wg_out[:, :]],
            )

            boff_regs = nc.alloc_registers("boff_regs",
                                           engines=(mybir.EngineType.DVE,))
            nc.regs_load(boff_regs, boff_sb[0:1, 0:1])
            off_sv = nc.snap(boff_regs, donate=True, min_val=C,
                             max_val=(NCORES - 1) * 9 * C + C)
            boff2_regs = nc.alloc_registers("boff2_regs",
                                            engines=(mybir.EngineType.DVE,))
            nc.regs_load(boff2_regs, boff_sb[0:1, 1:2])
            own_sv = nc.snap(boff2_regs, donate=True, min_val=0,
                             max_val=(NCORES - 1) * 9 * C)

            # ---- helpers ----
            def softmax_pp(pool, u_ppv, mb, tag, out=None):
                v = u_ppv.rearrange("p (a c) -> p a c", c=C)
                mx = pool.tile([BLK, mb], f32, tag=f"{tag}_mx")
                nc.vector.tensor_reduce(out=mx, in_=v,
                                        axis=mybir.AxisListType.X,
                                        op=mybir.AluOpType.max)
                e = pool.tile([BLK, mb * C], f32, tag=f"{tag}_e")
                ev = e.rearrange("p (a c) -> p a c", c=C)
                nc.vector.tensor_sub(ev, v, bcast_inner(mx, C))
                nc.scalar.activation(out=e, in_=e, func=EXP)
                s = pool.tile([BLK, mb], f32, tag=f"{tag}_s")
                nc.vector.tensor_reduce(out=s, in_=ev,
                                        axis=mybir.AxisListType.X,
                                        op=mybir.AluOpType.add)
                nc.vector.reciprocal(out=s, in_=s)
                if out is None:
                    out = pool.tile([BLK, mb * C], f32, tag=f"{tag}_fl")
                nc.vector.tensor_mul(out.rearrange("p (a c) -> p a c", c=C),
                                     ev, bcast_inner(s, C))
                return out

            # ---- phase 2: initial flat = softmax(input), replicated ----
            with tc.tile_pool(name="init", bufs=1) as ipool:
                fl0 = softmax_pp(ipool, ipp_sb, GBLK, "sm0")
                nc.vector.tensor_copy(
                    out=flat_pad[:, PADBLK * C:(PADBLK + GBLK) * C], in_=fl0)
                nc.vector.tensor_copy(out=own32,
                                      in_=fl0[:, bass.ds(own_sv, 9 * C)])
                ohl = own_hl.rearrange("p (b t) -> p b t", t=HLW)
                o32v = own32.rearrange("p (b c) -> p b c", c=C)
                nc.vector.tensor_copy(out=ohl[:, :, 0:C], in_=o32v)
                nc.vector.tensor_sub(ohl[:, :, 32:32 + C], o32v,
                                     ohl[:, :, 0:C])

            # ---- phase 1: build K band (fp16) ----
            with tc.tile_pool(name="sp3p", bufs=2) as sp3pool, \
                 tc.tile_pool(name="ep", bufs=2) as epool, \
                 tc.tile_pool(name="p1ps", bufs=2, space="PSUM") as p1pool:
                for ci in range(3):
                    lo_b, hi_b = BAND[ci]
                    nb, ap = NBLKC[ci], CHUNKS[ci][2]
                    e0 = (CHUNKS[ci][0] + 1) * W
                    sp3_sb = sp3pool.tile([BLK, 34 * 480], f16, tag="sp3")
                    spq = [0, 8 * ap, 16 * ap, 24 * ap, nb * ap]
                    for q in range(4):
                        nc.sync.dma_start(
                            out=sp3_sb[:, spq[q]:spq[q + 1]],
                            in_=sp3_dram[:, CBASE[ci] + spq[q]:
                                         CBASE[ci] + spq[q + 1]])
                    ktv = kt[ci].rearrange("p (j a) -> p j a", a=ap)
                    sp3v = sp3_sb[:, 0:nb * ap].rearrange(
                        "p (j a) -> p j a", a=ap)
                    for j0 in range(0, nb, ACTB):
                        nj = min(ACTB, nb - j0)
                        pb = p1pool.tile([BLK, ACTB, 512], f32, tag="pb")
                        for jj in range(nj):
                            bi = lo_b - BAND_LO + j0 + jj
                            nc.tensor.matmul(
                                pb[:, jj, 0:ap],
                                g_sb[:, bi * BLK:(bi + 1) * BLK],
                                h_sb[:, e0:e0 + ap],
                                start=True, stop=True)
                        # E = exp(d2y+d2x+d2c+ln3) straight into K
                        nc.scalar.activation(out=ktv[:, j0:j0 + nj, :],
                                             in_=pb[:, 0:nj, 0:ap],
                                             func=EXP, bias=ln3_sb)
                        # K += SP3 (precomputed 3*gy*gx, fp16)
                        nc.vector.tensor_add(ktv[:, j0:j0 + nj, :],
                                             ktv[:, j0:j0 + nj, :],
                                             sp3v[:, j0:j0 + nj, :])

            # ---- phase 3: iterations ----
            with tc.tile_pool(name="smx", bufs=2) as spool, \
                 tc.tile_pool(name="ipsum", bufs=1, space="PSUM") as ippool, \
                 tc.tile_pool(name="tpsum", bufs=2, space="PSUM") as tppool:
                pv = [ippool.tile([HLW, 512], f32, tag=f"pv{ci}",
                                  name=f"pv{ci}") for ci in range(3)]

                def src(bi):
                    if OWN_LO <= bi < OWN_HI:
                        o = bi - OWN_LO
                        return own_hl[:, o * HLW:(o + 1) * HLW]
                    return band_hl[:, bi * HLW:(bi + 1) * HLW]

                ktvs = [kt[ci].rearrange("p (j a) -> p j a",
                                         a=CHUNKS[ci][2]) for ci in range(3)]

                def chunks_for(b):
                    return [ci for ci in range(3)
                            if BAND[ci][0] <= b < BAND[ci][1]]

                for it in range(ITERS):
                    tot = [BAND[ci][1] - BAND[ci][0] for ci in range(3)]
                    cnt = [0, 0, 0]

                    def emit_mm(ci, b):
                        bi = b - BAND_LO
                        j = b - BAND[ci][0]
                        ap = CHUNKS[ci][2]
                        cnt[ci] += 1
                        nc.tensor.matmul(pv[ci][:, 0:ap], src(bi),
                                         ktvs[ci][:, j, :],
                                         start=cnt[ci] == 1,
                                         stop=cnt[ci] == tot[ci])

                    # own blocks first (data ready pre-gather)
                    for b in range(0, 9):
                        for ci in chunks_for(b):
                            emit_mm(ci, b)
                    # band tiles from gathered flat (stalls until scatter)
                    bhl = band_hl.rearrange("p (b t) -> p b t", t=HLW)
                    pdv = flat_pad[:, bass.ds(off_sv, NBLK_U * C)].rearrange(
                        "p (b c) -> p b c", c=C)
                    nc.vector.tensor_copy(out=bhl[:, :, 0:C], in_=pdv)
                    nc.vector.tensor_sub(bhl[:, :, 32:32 + C], pdv,
                                         bhl[:, :, 0:C])
                    # far blocks chunk-sequential: chunk ci's psum closes
                    # while ci+1's matmuls still run -> x-pass overlaps PE
                    for ci in range(3):
                        lo_r, hi_r, ap = CHUNKS[ci]
                        for b in range(BAND[ci][0], BAND[ci][1]):
                            if not (0 <= b < 9):
                                emit_mm(ci, b)
                        # PSUM -> SBUF (scalar), merge lo rows, then x-pass
                        e0 = (lo_r + 1) * W
                        cb = comb_sb[:, e0:e0 + ap]
                        nc.scalar.activation(out=cb, in_=pv[ci][0:C, 0:ap],
                                             func=COPY)
                        nc.vector.tensor_add(cb, cb,
                                             pv[ci][32:32 + C, 0:ap])
                        t1c = comb_t1[:, e0:e0 + ap]
                        nc.vector.tensor_add(t1c[:, 1:ap - 1], cb[:, 0:ap - 2],
                                             cb[:, 2:ap])
                        nc.vector.tensor_add(t1c[:, 1:ap - 1], t1c[:, 1:ap - 1],
                                             cb[:, 1:ap - 1])
                        t1r = t1c.rearrange("p (r x) -> p r x", x=W)
                        cbr = cb.rearrange("p (r x) -> p r x", x=W)
                        nc.vector.tensor_add(t1r[:, :, 0:1], cbr[:, :, 0:1],
                                             cbr[:, :, 1:2])
                        nc.vector.tensor_add(t1r[:, :, 0:1], t1r[:, :, 0:1],
                                             cbr[:, :, 0:1])
                        nc.vector.tensor_add(t1r[:, :, W - 1:W],
                                             cbr[:, :, W - 2:W - 1],
                                             cbr[:, :, W - 1:W])
                        nc.vector.tensor_add(t1r[:, :, W - 1:W],
                                             t1r[:, :, W - 1:W],
                                             cbr[:, :, W - 1:W])
                        # y-pass A after chunk 1 (u rows 0..7)
                        if ci == 1:
                            nc.vector.tensor_add(u_cn[:, 0:768],
                                                 comb_t1[:, 0:768],
                                                 comb_t1[:, W:768 + W])
                            nc.vector.tensor_add(u_cn[:, 0:768],
                                                 u_cn[:, 0:768],
                                                 comb_t1[:, 2 * W:768 + 2 * W])
                            nc.vector.tensor_add(u_cn[:, 0:768],
                                                 u_cn[:, 0:768],
                                                 icn_sb[:, 0:768])
                            for kb in range(6):
                                pt = tppool.tile([BLK, C], f32, tag="pt")
                                nc.tensor.transpose(
                                    pt, u_cn[:, kb * BLK:(kb + 1) * BLK],
                                    ident[0:C, 0:C])
                                nc.vector.tensor_copy(
                                    out=u_pp[:, kb * C:(kb + 1) * C], in_=pt)
                    # y-pass B (u rows 8..11) + remaining transposes
                    nc.vector.tensor_add(u_cn[:, 768:NMID],
                                         comb_t1[:, 768:NMID],
                                         comb_t1[:, 768 + W:NMID + W])
                    nc.vector.tensor_add(u_cn[:, 768:NMID], u_cn[:, 768:NMID],
                                         comb_t1[:, 768 + 2 * W:NMID + 2 * W])
                    nc.vector.tensor_add(u_cn[:, 768:NMID], u_cn[:, 768:NMID],
                                         icn_sb[:, 768:NMID])
                    for kb in range(6, 9):
                        pt = tppool.tile([BLK, C], f32, tag="pt")
                        nc.tensor.transpose(pt,
                                            u_cn[:, kb * BLK:(kb + 1) * BLK],
                                            ident[0:C, 0:C])
                        nc.vector.tensor_copy(
                            out=u_pp[:, kb * C:(kb + 1) * C], in_=pt)

                    # softmax split: blocks 0-5 ready before 6-8
                    softmax_pp(spool, u_pp[:, 0:6 * C], 6, "smxA",
                               out=own32[:, 0:6 * C])
                    softmax_pp(spool, u_pp[:, 6 * C:9 * C], 3, "smxB",
                               out=own32[:, 6 * C:9 * C])

                    if it < ITERS - 1:
                        nc.sync.dma_start(out=ag_in, in_=own32)
                        nc.gpsimd.collective_compute(
                            "AllGather", mybir.AluOpType.bypass,
                            replica_groups=[list(range(NCORES))],
                            ins=[ag_in.opt()], outs=[ag_out[:, :]],
                        )
                        nc.sync.dma_start(
                            out=flat_pad[:, PADBLK * C:(PADBLK + GBLK) * C]
                            .rearrange("p (r j) -> p r j", r=NCORES),
                            in_=ag_out[:, :].rearrange("(r p) j -> p r j",
                                                       p=BLK))
                    else:
                        nc.sync.dma_start(out=out_dram[:, :], in_=own32)
                    # own hi/lo for the NEXT iter: off the pre-gather path
                    ohl2 = own_hl.rearrange("p (b t) -> p b t", t=HLW)
                    o32v2 = own32.rearrange("p (b c) -> p b c", c=C)
                    nc.vector.tensor_copy(out=ohl2[:, :, 0:C], in_=o32v2)
                    nc.vector.tensor_sub(ohl2[:, :, 32:32 + C], o32v2,
                                         ohl2[:, :, 0:C])

    nc.compile()
    return nc


def _host_inputs(input_tensor, reference_tensor):
    import ml_dtypes
    bf = ml_dtypes.bfloat16

    logits = np.ascontiguousarray(
        np.asarray(input_tensor, dtype=np.float32)[0].reshape(C, N))
    ref = np.asarray(reference_tensor, dtype=np.float32)[0]  # [3, 96, 96]

    yy = (np.arange(N) // W).astype(np.float64)
    xx = (np.arange(N) % W).astype(np.float64)
    xp = xx - 47.5
    cc = ref.reshape(3, N).astype(np.float64) / 0.5
    ones = np.ones(N, np.float64)

    def hi_lo(x):
        h = np.asarray(x, np.float64).astype(bf).astype(np.float64)
        l = (np.asarray(x, np.float64) - h).astype(bf).astype(np.float64)
        return h, l

    def hi_lo3(x):
        x = np.asarray(x, np.float64)
        h = x.astype(bf).astype(np.float64)
        l = (x - h).astype(bf).astype(np.float64)
        l2 = (x - h - l).astype(bf).astype(np.float64)
        return h, l, l2

    # feature rows (G paired with H), d2 = -(dy^2)/50 - 0.5*|dc~|^2
    def feat_rows(yp):
        rows_G, rows_H = [], []

        def pair(g, h):
            rows_G.append(np.asarray(g, np.float64))
            rows_H.append(np.asarray(h, np.float64))

        g_h, g_l = hi_lo(-yp * yp / 50.0)
        pair(g_h, ones); pair(g_l, ones)
        h_h, h_l = hi_lo(yp / 25.0)
        pair(yp, h_h); pair(yp, h_l)
        h_h, h_l = hi_lo(-yp * yp / 50.0)
        pair(ones, h_h); pair(ones, h_l)
        a, b, c2_ = hi_lo3(-xp * xp / 50.0)
        pair(a, ones); pair(b, ones); pair(c2_, ones)
        a, b, c2_ = hi_lo3(xp / 25.0)
        pair(xp, a); pair(xp, b); pair(xp, c2_)
        a, b, c2_ = hi_lo3(-xp * xp / 50.0)
        pair(ones, a); pair(ones, b); pair(ones, c2_)
        for ch in range(3):
            cm_h, cm_l = hi_lo(cc[ch])
            pair(cm_h, cm_h); pair(cm_h, cm_l); pair(cm_l, cm_h)
        csq = -0.5 * (cc * cc).sum(axis=0)
        g_h, g_l = hi_lo(csq)
        pair(g_h, ones); pair(g_l, ones)
        pair(ones, g_h); pair(ones, g_l)
        return np.stack(rows_G), np.stack(rows_H)  # [19, N] each

    dtab = np.exp(-(np.arange(-(H + 32), H + 32) ** 2) / 50.0)
    yy_i = (np.arange(N) // W).astype(np.int64)
    xx_i = (np.arange(N) % W).astype(np.int64)

    # ipp: logits in pixel-partition layout [128, 72*5]
    ipp = np.ascontiguousarray(
        logits.reshape(C, GBLK, BLK).transpose(2, 1, 0).reshape(BLK, GBLK * C))

    in_maps = []
    for r in range(NCORES):
        yc = 12 * r + 6
        G_all, H_all = feat_rows(yy - yc)
        yext = np.clip(np.arange(RPC * r - 1, RPC * (r + 1) + 1), 0, H - 1)
        hpix = (yext[:, None] * W + np.arange(W)[None, :]).reshape(-1)

        g = np.zeros((NFEAT, NBLK_U * BLK), np.float64)
        g[0, :] = NEG
        for bi in range(NBLK_U):
            gb = 9 * r + BAND_LO + bi
            if 0 <= gb < GBLK:
                g[:, bi * BLK:(bi + 1) * BLK] = \
                    G_all[:, gb * BLK:(gb + 1) * BLK]
        h = H_all[:, hpix]

        sp3 = np.zeros((BLK, KW), np.float16)
        for ci, (lo_r, hi_r, ap) in enumerate(CHUNKS):
            nrows = hi_r - lo_r
            yn = yext[(lo_r + 1):(lo_r + 1) + nrows]
            for j in range(NBLKC[ci]):
                gb = 9 * r + BAND[ci][0] + j
                if not (0 <= gb < GBLK):
                    continue
                pm = np.arange(gb * BLK, (gb + 1) * BLK)
                A = 3.0 * dtab[yy_i[pm][:, None] - yn[None, :] + H + 32]
                B = dtab[xx_i[pm][:, None] - np.arange(W)[None, :] + H + 32]
                blkv = (A[:, :, None] * B[:, None, :]).reshape(BLK, ap)
                c0 = CBASE[ci] + j * ap
                sp3[:, c0:c0 + ap] = blkv.astype(np.float16)

        icn = np.ascontiguousarray(
            logits.reshape(C, H, W)[:, RPC * r:RPC * (r + 1), :]
            .reshape(C, NMID))
        in_maps.append({
            "g_feats": np.ascontiguousarray(g).astype(bf),
            "h_feats": np.ascontiguousarray(h).astype(bf),
            "sp3": sp3,
            "inp_pp": ipp,
            "inp_cn": icn,
            "band_off": np.array([[(PADBLK + 9 * r + BAND_LO) * C, 9 * C * r]], np.uint32),
        })
    return in_maps


def _assemble(results):
    out = np.empty((C, N), np.float32)
    for r in range(NCORES):
        blk = results[r]["out_loc"].reshape(BLK, NMID // BLK, C)
        out[:, NMID * r:NMID * (r + 1)] = (
            blk.transpose(2, 1, 0).reshape(C, NMID))
    return out.reshape(1, C, H, W)


def _get_nc():
    global _CACHED_NC
    if _CACHED_NC is None:
        _CACHED_NC = _build_module()
    return _CACHED_NC


def run(input_tensor, reference_tensor, trace=False):
    from concourse.bass_utils import run_bass_kernel_spmd
    nc = _get_nc()
    in_maps = _host_inputs(input_tensor, reference_tensor)
    res = run_bass_kernel_spmd(nc, in_maps, core_ids=list(range(NCORES)),
                               trace=trace)
    return _assemble(res.results), res


def kernel(input_tensor, reference_tensor):
    out, _ = run(input_tensor, reference_tensor, trace=False)
    return out
